# revision 3
# baseline (speedup 1.0000x reference)
"""Trainium2 Bass kernel for KeypointAlignmentLossL2.

Strategy (data-parallel over batch, one NeuronCore per batch element):
  Host prep (per core b):
    - transpose feat[b] from [C, H*W] to pixel-major [H*W, C], cast bf16
    - compute bilinear corner indices / weights from kp[b] (f32, exact
      floor/sub semantics; x0 clamped to W-2 with wx in [0,1] so all four
      corners are always in-bounds — identical math to the reference's
      zero-padded gather for coords in [0, W-1])
    - weights are packed as 128x128 bf16 diagonal matrices so the lerp can
      run on the tensor engine as accumulating diagonal matmuls
  Device (per core):
    - dma_gather: 4 corner rows (768 ch, bf16) per keypoint straight from
      HBM into SBUF, keypoint -> partition
    - TensorE: f = sum_nb diag(w_nb) @ g_nb accumulated in PSUM (f32)
    - ScalarE: copy f PSUM->SBUF
    - VectorE: fused tensor_tensor_reduce for ||f1||^2, ||f2||^2, <f1,f2>
    - outputs three [128, 8] f32 tiles (keypoint-chunk layout)
  Host finish: masked mean of 2 - 2*cos distances across all cores.
"""
import numpy as np
import ml_dtypes

B, C, H, W, N = 8, 768, 64, 64, 1024
HW_ = H * W
NCHUNK = N // 128  # 8 chunks of 128 keypoints
NQ = 4             # gather calls per image; each covers 2 chunks (1024 idxs)

_CACHE = {}


def _build_nc():
    from contextlib import ExitStack
    import concourse.bass as bass
    import concourse.tile as tile
    import concourse.mybir as mybir
    from concourse import bacc

    f32 = mybir.dt.float32
    bf16 = mybir.dt.bfloat16
    i16 = mybir.dt.int16

    nc = bacc.Bacc("TRN2", target_bir_lowering=False, debug=False, num_devices=8)

    featT1 = nc.dram_tensor("featT1", [HW_, C], bf16, kind="ExternalInput")
    featT2 = nc.dram_tensor("featT2", [HW_, C], bf16, kind="ExternalInput")
    idx1 = nc.dram_tensor("idx1", [128, 4 * N // 16], i16, kind="ExternalInput")
    idx2 = nc.dram_tensor("idx2", [128, 4 * N // 16], i16, kind="ExternalInput")
    wd = nc.dram_tensor("wd", [128, 2 * NCHUNK * 4, 128], bf16, kind="ExternalInput")
    out_n1 = nc.dram_tensor("out_n1", [128, NCHUNK], f32, kind="ExternalOutput")
    out_n2 = nc.dram_tensor("out_n2", [128, NCHUNK], f32, kind="ExternalOutput")
    out_dot = nc.dram_tensor("out_dot", [128, NCHUNK], f32, kind="ExternalOutput")

    featTs = (featT1, featT2)
    idxs_dram = (idx1, idx2)
    MULT = mybir.AluOpType.mult
    ADD = mybir.AluOpType.add

    with tile.TileContext(nc) as tc, ExitStack() as ctx:
        const_pool = ctx.enter_context(tc.tile_pool(name="const", bufs=1))
        gpool = ctx.enter_context(tc.tile_pool(name="g", bufs=4))
        fpool = ctx.enter_context(tc.tile_pool(name="f", bufs=4))
        dpool = ctx.enter_context(tc.tile_pool(name="d", bufs=2))
        ppool = ctx.enter_context(
            tc.tile_pool(name="p", bufs=8, space=bass.MemorySpace.PSUM)
        )

        wd_t = const_pool.tile([128, 2 * NCHUNK * 4, 128], bf16, tag="wd")
        nc.sync.dma_start(wd_t[:], wd[:])
        idx_t = []
        for im in range(2):
            t = const_pool.tile([128, 4 * N // 16], i16, tag=f"idx{im}", name=f"idx{im}")
            nc.sync.dma_start(t[:], idxs_dram[im][:])
            idx_t.append(t)

        res = []
        for name in ("n1", "n2", "dot"):
            res.append(const_pool.tile([128, NCHUNK], f32, tag=f"res_{name}", name=f"res_{name}"))

        for q in range(NQ):
            gt = []
            for im in range(2):
                g = gpool.tile([128, 2 * 4, C], bf16, tag="g")
                nc.gpsimd.dma_gather(
                    g[:],
                    featTs[im][:],
                    idx_t[im][:, q * 64:(q + 1) * 64],
                    1024,
                    1024,
                    C,
                )
                gt.append(g)
            for j in range(2):
                ch = 2 * q + j
                fs = []
                for im in range(2):
                    f_sb = fpool.tile([128, C], f32, tag="f")
                    for h in range(2):
                        ps = ppool.tile([128, C // 2], f32, tag="ps")
                        for nb in range(4):
                            nc.tensor.matmul(
                                ps[:],
                                wd_t[:, (im * NCHUNK + ch) * 4 + nb, :],
                                gt[im][:, 4 * j + nb, h * (C // 2):(h + 1) * (C // 2)],
                                start=(nb == 0),
                                stop=(nb == 3),
                            )
                        nc.scalar.copy(f_sb[:, h * (C // 2):(h + 1) * (C // 2)], ps[:])
                    fs.append(f_sb)
                dump_a = dpool.tile([128, C], f32, tag="dump_a", name="dump_a")
                dump_b = dpool.tile([128, C], f32, tag="dump_b", name="dump_b")
                nc.scalar.activation(
                    dump_a[:], fs[0][:], mybir.ActivationFunctionType.Square,
                    accum_out=res[0][:, ch:ch + 1],
                )
                nc.scalar.activation(
                    dump_a[:], fs[1][:], mybir.ActivationFunctionType.Square,
                    accum_out=res[1][:, ch:ch + 1],
                )
                nc.vector.tensor_tensor(dump_b[:], fs[0][:], fs[1][:], op=MULT)
                nc.vector.tensor_reduce(
                    res[2][:, ch:ch + 1], dump_b[:],
                    axis=mybir.AxisListType.X, op=ADD,
                )

        nc.sync.dma_start(out_n1[:], res[0][:])
        nc.sync.dma_start(out_n2[:], res[1][:])
        nc.sync.dma_start(out_dot[:], res[2][:])

    nc.compile()
    return nc


def get_nc():
    if "nc" not in _CACHE:
        _CACHE["nc"] = _build_nc()
    return _CACHE["nc"]


def _host_prep_img(feat_b, kp_b):
    """feat_b [C,H,W] f32, kp_b [N,2] f32 ->
    featT bf16 [HW_, C], nb_idx int32 [4, N], w f32 [4, N]"""
    featT = np.ascontiguousarray(
        np.asarray(feat_b, np.float32).reshape(C, HW_).T
    ).astype(ml_dtypes.bfloat16)
    x = np.asarray(kp_b[:, 0], np.float32)
    y = np.asarray(kp_b[:, 1], np.float32)
    x0 = np.minimum(np.floor(x), np.float32(W - 2)).astype(np.float32)
    y0 = np.minimum(np.floor(y), np.float32(H - 2)).astype(np.float32)
    wx = (x - x0).astype(np.float32)
    wy = (y - y0).astype(np.float32)
    pix = y0.astype(np.int32) * W + x0.astype(np.int32)
    nb_idx = np.stack([pix, pix + 1, pix + W, pix + W + 1], 0)
    w = np.stack(
        [(1 - wx) * (1 - wy), wx * (1 - wy), (1 - wx) * wy, wx * wy], 0
    ).astype(np.float32)
    return featT, nb_idx, w


def _make_idx_layout(nb_idx):
    """[4,N] corner indices -> [128, 4N/16] int16 SBUF index layout
    (element i=(4*ch+nb)*128+p lives at [i%16 (replicated x8), i//16])."""
    unwrapped = nb_idx.reshape(4, NCHUNK, 128).transpose(1, 0, 2).reshape(-1)
    lay = unwrapped.reshape(-1, 16).T
    return np.tile(lay, (8, 1)).astype(np.int16)


def _make_wd(w1, w2):
    """weights [4,N] f32 per image -> [128, 64, 128] bf16 diagonal matrices"""
    wd = np.zeros((128, 2 * NCHUNK * 4, 128), np.float32)
    r = np.arange(128)
    for im, w in ((0, w1), (1, w2)):
        for ch in range(NCHUNK):
            for nb in range(4):
                k = (im * NCHUNK + ch) * 4 + nb
                wd[r, k, r] = w[nb, ch * 128:(ch + 1) * 128]
    return wd.astype(ml_dtypes.bfloat16)


def kernel(feat1, feat2, kp1, kp2, kp1_mask, kp2_mask):
    from concourse.bass_utils import run_bass_kernel_spmd

    feat1 = np.asarray(feat1, np.float32)
    feat2 = np.asarray(feat2, np.float32)
    kp1 = np.asarray(kp1, np.float32)
    kp2 = np.asarray(kp2, np.float32)
    kp1_mask = np.asarray(kp1_mask)
    kp2_mask = np.asarray(kp2_mask)

    nc = get_nc()
    in_maps = []
    for b in range(B):
        fT1, nb1, w1 = _host_prep_img(feat1[b], kp1[b])
        fT2, nb2, w2 = _host_prep_img(feat2[b], kp2[b])
        in_maps.append({
            "featT1": fT1,
            "featT2": fT2,
            "idx1": _make_idx_layout(nb1),
            "idx2": _make_idx_layout(nb2),
            "wd": _make_wd(w1, w2),
        })

    results = run_bass_kernel_spmd(nc, in_maps, list(range(B))).results

    sum_l2 = 0.0
    sum_valid = 0.0
    for b in range(B):
        r = results[b]
        n1sq = r["out_n1"].T.reshape(-1).astype(np.float64)
        n2sq = r["out_n2"].T.reshape(-1).astype(np.float64)
        dot = r["out_dot"].T.reshape(-1).astype(np.float64)
        m1 = np.maximum(np.sqrt(n1sq), 1e-12)
        m2 = np.maximum(np.sqrt(n2sq), 1e-12)
        l2 = n1sq / (m1 * m1) + n2sq / (m2 * m2) - 2.0 * dot / (m1 * m2)
        valid = (kp1_mask[b] & kp2_mask[b]).astype(np.float64)
        sum_l2 += float((l2 * valid).sum())
        sum_valid += float(valid.sum())

    loss = 0.0 if sum_valid == 0 else sum_l2 / max(sum_valid, 1.0)
    return np.float32(loss)
